# revision 11
# baseline (speedup 1.0000x reference)
"""ChebyshevGCN Trainium2 kernel: 8-core row-parallel SpMM with per-step AllGather.

Math (per layer l in 0..1, poly order K=10):
    lap = -adj/deg[:,None]                     [N, N], N=8192
    Z_0 = X; Z_1 = lap@X; Z_k = 2*lap@Z_{k-1} - Z_{k-2}
    X = tanh(sum_k Z_k @ W[l,k] + b[l])

Distribution: core r owns output rows r*1024..(r+1)*1024. Each core keeps the
bf16 transpose of its lap row-block (lapT column block, [8192, 1024]) resident
in SBUF and computes its row block of lap@Z each step. Z is all-gathered in
bf16 twice per step in asymmetric 5/3 row-chunk halves: the small second
gather is consumed last in the next step's matmul sweep, hiding the ~20us
collective latency. Y = sum_k Z_k W_k accumulates directly in pinned PSUM
banks across the whole layer. bf16 inputs with fp32 PSUM accumulation were
validated bit-exact against the fp32 reference (the network saturates tanh).
"""

import os
import sys
from contextlib import ExitStack

for _p in ("/opt/trn_rl_repo", "/root/.axon_site/_ro/trn_rl_repo"):
    if os.path.isdir(_p) and _p not in sys.path:
        sys.path.append(_p)

import numpy as np
import ml_dtypes

from concourse import bacc, tile, bass_utils, mybir
from concourse.bass import _add_dep_helper

BF16 = ml_dtypes.bfloat16

N = 8192          # nodes
D = 256           # width
NCORES = 8
ROWS = N // NCORES          # 1024 local rows
P = 128                     # partitions
IC = ROWS // P              # 8 local row chunks
JC = N // P                 # 64 contraction chunks
KPOLY = 10
NLAYERS = 2
SPLITS = (5, 3)             # row chunks per half-step gather
OFFS = (0, 5)

_BUILT = None


def _build():
    nc = bacc.Bacc("TRN2", target_bir_lowering=False, debug=False,
                   num_devices=NCORES)
    f32 = mybir.dt.float32
    bf = mybir.dt.bfloat16

    bp_d = nc.dram_tensor("bp", [N, ROWS], bf, kind="ExternalInput").ap()
    # X pre-shuffled into the gathered layout used by every step:
    # xg[h][r*128+p, q*256+d] = X[r*1024 + (OFFS[h]+q)*128 + p, d]
    xg0_d = nc.dram_tensor("xg0", [NCORES * P, SPLITS[0] * D], bf, kind="ExternalInput").ap()
    xg1_d = nc.dram_tensor("xg1", [NCORES * P, SPLITS[1] * D], bf, kind="ExternalInput").ap()
    xloc_d = nc.dram_tensor("xloc", [ROWS, D], bf, kind="ExternalInput").ap()
    xt_d = nc.dram_tensor("xt", [D, ROWS], bf, kind="ExternalInput").ap()
    w_d = nc.dram_tensor("w", [NLAYERS * KPOLY * 2, P, D], bf, kind="ExternalInput").ap()
    b_d = nc.dram_tensor("b", [NLAYERS, ROWS, D], f32, kind="ExternalInput").ap()
    id_d = nc.dram_tensor("ident", [P, P], bf, kind="ExternalInput").ap()
    out_d = nc.dram_tensor("out", [ROWS, D], f32, kind="ExternalOutput").ap()

    rg = [list(range(NCORES))]
    COPY = mybir.ActivationFunctionType.Copy
    TANH = mybir.ActivationFunctionType.Tanh
    MUL = mybir.AluOpType.mult
    SUB = mybir.AluOpType.subtract
    ADD = mybir.AluOpType.add

    with tile.TileContext(nc) as tc, ExitStack() as ctx:
        bppool = ctx.enter_context(tc.tile_pool(name="bp", bufs=JC))
        cstpool = ctx.enter_context(tc.tile_pool(name="cst", bufs=1))
        zlpool = ctx.enter_context(tc.tile_pool(name="zl", bufs=6))
        ztpool = ctx.enter_context(tc.tile_pool(name="zt", bufs=2))
        zspool = ctx.enter_context(tc.tile_pool(name="zs", bufs=5))
        tmppool = ctx.enter_context(tc.tile_pool(name="tmp", bufs=2))
        ocpool = ctx.enter_context(tc.tile_pool(name="oc", bufs=2))
        pspool = ctx.enter_context(tc.tile_pool(name="ps", bufs=4, space="PSUM"))
        ypool = ctx.enter_context(tc.tile_pool(name="y", bufs=1, space="PSUM"))
        dram = ctx.enter_context(tc.tile_pool(name="dram", bufs=8, space="DRAM"))

        # ---- constants / small residents (cheap; issued first) ----
        w_sb = cstpool.tile([P, NLAYERS * KPOLY * 2, D], bf, name="w_sb")
        nc.sync.dma_start(w_sb[:], w_d.rearrange("m p e -> p m e"))
        idn = cstpool.tile([P, P], bf, name="idn")
        nc.sync.dma_start(idn[:], id_d[:])
        zloc_prev1 = []
        for h in range(2):
            t = zlpool.tile([P, SPLITS[0], D], bf, name=f"zloc0_{h}", tag="zloc")
            nc.sync.dma_start(
                t[:, :SPLITS[h], :],
                xloc_d.rearrange("(c p) d -> p c d", p=P)[:, OFFS[h]:OFFS[h] + SPLITS[h], :])
            zloc_prev1.append(t)
        zt_cur = ztpool.tile([P, 2, ROWS], bf, name="xt0", tag="zt")
        nc.sync.dma_start(zt_cur[:], xt_d.rearrange("(dc p) i -> p dc i", p=P))

        # bp chunks are DMA'd on first use so the 16MB resident load paces
        # with the first step's matmul sweep instead of serializing ahead.
        bp_src = bp_d.rearrange("(c p) i -> p c i", p=P)
        bp_sb = {}

        def get_bp(jc):
            if jc not in bp_sb:
                t = bppool.tile([P, ROWS], bf, name=f"bp{jc}", tag="bp")
                nc.sync.dma_start(t[:], bp_src[:, jc, :])
                bp_sb[jc] = t
            return bp_sb[jc]

        b_sb_holder = []

        def get_b():
            if not b_sb_holder:
                t = cstpool.tile([P, NLAYERS, IC, D], f32, name="b_sb")
                nc.sync.dma_start(t[:], b_d.rearrange("l (c p) d -> p l c d", p=P))
                b_sb_holder.append(t)
            return b_sb_holder[0]

        def y_accum(Y, zt_t, l, k, ydeps, ics=range(IC)):
            # Y[:, ic, :] accumulates in pinned PSUM across the whole layer.
            # start clears has_written for a whole bank, so only the very
            # first matmul touching each bank (ic even, k==0, dc==0) sets it;
            # the odd-ic first matmul is ordered after it explicitly.
            for ic in ics:
                m = (l * KPOLY + k) * 2
                for dc in range(2):
                    mm = nc.tensor.matmul(
                        Y[:, ic, :], lhsT=zt_t[:, dc, ic * P:(ic + 1) * P],
                        rhs=w_sb[:, m + dc, :],
                        start=(k == 0 and dc == 0 and ic % 2 == 0),
                        stop=(k == KPOLY - 1 and dc == 1 and ic % 2 == 1),
                        skip_group_check=True)
                    if k == 0 and dc == 0:
                        if ic % 2 == 0:
                            ydeps[ic // 2] = mm
                        else:
                            _add_dep_helper(mm.ins, ydeps[ic // 2].ins, False,
                                            "bank-clear start runs first")

        def transpose_ics(zt_t, src_h, ics):
            # [128,128] bf16 transposes via the DMA xbar (off the PE)
            for ic in ics:
                h = 0 if ic < SPLITS[0] else 1
                q = ic - OFFS[h]
                for dc in range(2):
                    nc.scalar.dma_start_transpose(
                        zt_t[:, dc, ic * P:(ic + 1) * P],
                        src_h[h][:, q, dc * P:(dc + 1) * P])

        def transpose_into(zt_t, src_h, l, k):
            transpose_ics(zt_t, src_h, range(IC))

        def gather(zloc_h, l, k, h):
            ns = SPLITS[h]
            agi = dram.tile([P, ns * D], bf, name=f"agi{l}_{k}_{h}", tag=f"agi{h}")
            nc.sync.dma_start(agi[:], zloc_h[:, :ns, :].rearrange("p c d -> p (c d)"))
            ago = dram.tile([NCORES * P, ns * D], bf, addr_space="Shared",
                            name=f"ago{l}_{k}_{h}", tag=f"ago{h}")
            nc.gpsimd.collective_compute(
                "AllGather", mybir.AluOpType.bypass, replica_groups=rg,
                ins=[agi[:].opt()], outs=[ago[:].opt()])
            return ago

        agout_prev = None  # layer 0 step 1 reads xg from DRAM directly
        zloc_prev2 = None

        for l in range(NLAYERS):
            Y = ypool.tile([P, IC, D], f32, name=f"y{l}", tag="y")
            ydeps = {}
            y_accum(Y, zt_cur, l, 0, ydeps)

            for k in range(1, KPOLY):
                if k == KPOLY - 2:
                    b_sb = get_b()
                zloc_k = [zlpool.tile([P, SPLITS[0], D], bf, name=f"zloc{l}_{k}_{h}",
                                      tag="zloc") for h in range(2)]
                zt_k = ztpool.tile([P, 2, ROWS], bf, name=f"zt{l}_{k}", tag="zt")
                if k == KPOLY - 1:
                    # layer tail is finalized per half so the boundary
                    # gathers/output overlap the second half's matmul sweep
                    if l == 0:
                        x1 = [zlpool.tile([P, SPLITS[0], D], bf, name=f"x1loc_{h}",
                                          tag="zloc") for h in range(2)]
                        xt1 = ztpool.tile([P, 2, ROWS], bf, name="xt1", tag="zt")
                agout_k = [None, None]
                for half in range(2):
                    ns = SPLITS[half]
                    npair = (ns + 1) // 2
                    ps = [pspool.tile([P, 2, D], f32, name=f"psr{l}_{k}_{half}_{t}",
                                      tag="ps") for t in range(npair)]
                    firstmm = {}
                    nmm = 0
                    for sh in range(2):
                        for r in range(NCORES):
                            zs = zspool.tile([P, SPLITS[0], D], bf,
                                             name=f"zs{l}_{k}_{half}_{sh}_{r}", tag="zs")
                            if l == 0 and k == 1:
                                src = (xg0_d if sh == 0 else xg1_d)[r * P:(r + 1) * P, :]
                            else:
                                src = agout_prev[sh][r * P:(r + 1) * P, :]
                            nc.sync.dma_start(
                                zs[:, :SPLITS[sh], :].rearrange("p c d -> p (c d)"), src)
                            for q in range(SPLITS[sh]):
                                jc = r * IC + OFFS[sh] + q
                                bp_t = get_bp(jc)
                                nmm += 1
                                lastjc = nmm == JC
                                for u in range(ns):
                                    ic = OFFS[half] + u
                                    t, lane = u // 2, u % 2
                                    st = t not in firstmm
                                    mm = nc.tensor.matmul(
                                        ps[t][:, lane, :],
                                        lhsT=bp_t[:, ic * P:(ic + 1) * P],
                                        rhs=zs[:, q, :],
                                        start=st,
                                        stop=(lastjc and u == min(2 * t + 1, ns - 1)),
                                        skip_group_check=True)
                                    if st:
                                        firstmm[t] = mm
                                    elif nmm == 1 and lane == 1:
                                        _add_dep_helper(mm.ins, firstmm[t].ins, False,
                                                        "bank-clear start runs first")
                    for u in range(ns):
                        t, lane = u // 2, u % 2
                        if k == 1:
                            nc.scalar.activation(zloc_k[half][:, u, :],
                                                 ps[t][:, lane, :], COPY)
                        else:
                            nc.vector.scalar_tensor_tensor(
                                out=zloc_k[half][:, u, :], in0=ps[t][:, lane, :],
                                scalar=2.0, in1=zloc_prev2[half][:, u, :],
                                op0=MUL, op1=SUB)
                    if k < KPOLY - 1:
                        agout_k[half] = gather(zloc_k[half], l, k, half)
                    else:
                        ics_h = range(OFFS[half], OFFS[half] + ns)
                        transpose_ics(zt_k, zloc_k, ics_h)
                        y_accum(Y, zt_k, l, k, ydeps, ics_h)
                        for ic in ics_h:
                            tmp = tmppool.tile([P, D], f32, name=f"pre{l}_{ic}",
                                               tag="tmp")
                            nc.vector.scalar_tensor_tensor(
                                out=tmp[:], in0=Y[:, ic, :], scalar=1.0,
                                in1=b_sb[:, l, ic, :], op0=MUL, op1=ADD)
                            if l == 0:
                                nc.scalar.activation(
                                    x1[half][:, ic - OFFS[half], :], tmp[:], TANH)
                            else:
                                oc = ocpool.tile([P, D], f32, name=f"oc{ic}", tag="oc")
                                nc.scalar.activation(oc[:], tmp[:], TANH)
                                nc.sync.dma_start(
                                    out_d.rearrange("(c p) d -> p c d", p=P)[:, ic, :],
                                    oc[:])
                        if l == 0:
                            transpose_ics(xt1, x1, ics_h)
                            agout_k[half] = gather(x1[half], l, 99, half)
                if k < KPOLY - 1:
                    transpose_into(zt_k, zloc_k, l, k)
                    y_accum(Y, zt_k, l, k, ydeps)
                zloc_prev2, zloc_prev1 = zloc_prev1, zloc_k
                agout_prev = agout_k

            if l == 0:
                zloc_prev1 = x1
                zloc_prev2 = None
                zt_cur = xt1

    nc.compile()
    return nc


def _get_nc():
    global _BUILT
    if _BUILT is None:
        _BUILT = _build()
    return _BUILT


def kernel(X, adj_mat, degree, W, b):
    X = np.asarray(X, dtype=np.float32)
    adj_mat = np.asarray(adj_mat, dtype=np.float32)
    degree = np.asarray(degree, dtype=np.float32)
    W = np.asarray(W, dtype=np.float32)
    b = np.asarray(b, dtype=np.float32)

    nc = _get_nc()

    xbf = X.astype(BF16)
    # gathered layouts: xg{h}[r*128+p, q*256+d] = X[r*1024 + (OFFS[h]+q)*128 + p, d]
    x4 = xbf.reshape(NCORES, IC, P, D)              # [r, c, p, d]
    xgs = []
    for h in range(2):
        sl = x4[:, OFFS[h]:OFFS[h] + SPLITS[h]]     # [r, q, p, d]
        xgs.append(np.ascontiguousarray(
            sl.transpose(0, 2, 1, 3).reshape(NCORES * P, SPLITS[h] * D)))
    ident = np.eye(P, dtype=BF16)
    wm = np.ascontiguousarray(
        W.reshape(NLAYERS * KPOLY, 2, P, D).reshape(NLAYERS * KPOLY * 2, P, D)
    ).astype(BF16)

    in_maps = []
    for r in range(NCORES):
        rows = slice(r * ROWS, (r + 1) * ROWS)
        lap_blk = (-adj_mat[rows] / degree[rows, None]).astype(BF16)   # [ROWS, N]
        bp = np.ascontiguousarray(lap_blk.T)                           # [N, ROWS]
        xloc = xbf[rows]
        in_maps.append({
            "bp": bp,
            "xg0": xgs[0],
            "xg1": xgs[1],
            "xloc": np.ascontiguousarray(xloc),
            "xt": np.ascontiguousarray(xloc.T),
            "w": wm,
            "b": np.ascontiguousarray(b[:, rows, :]),
            "ident": ident,
        })

    res = bass_utils.run_bass_kernel_spmd(
        nc, in_maps, core_ids=list(range(NCORES)),
        trace=bool(int(os.environ.get("CHEB_TRACE", "0"))))
    kernel.last_exec_time_ns = res.exec_time_ns
    out = np.concatenate([res.results[r]["out"] for r in range(NCORES)], axis=0)
    return out


kernel.last_exec_time_ns = None


# revision 12
# speedup vs baseline: 1.4005x; 1.4005x over previous
"""ChebyshevGCN Trainium2 kernel: 8-core row-parallel SpMM with per-step AllGather.

Math (per layer l in 0..1, poly order K=10):
    lap = -adj/deg[:,None]                     [N, N], N=8192
    Z_0 = X; Z_1 = lap@X; Z_k = 2*lap@Z_{k-1} - Z_{k-2}
    X = tanh(sum_k Z_k @ W[l,k] + b[l])

Distribution: core r owns output rows r*1024..(r+1)*1024. Each core keeps the
bf16 transpose of its lap row-block (lapT column block, [8192, 1024]) resident
in SBUF and computes its row block of lap@Z each step. Z is all-gathered in
bf16 twice per step in asymmetric 5/3 row-chunk halves: the small second
gather is consumed last in the next step's matmul sweep, hiding the ~20us
collective latency. Y = sum_k Z_k W_k accumulates directly in pinned PSUM
banks across the whole layer. bf16 inputs with fp32 PSUM accumulation were
validated bit-exact against the fp32 reference (the network saturates tanh).
"""

import os
import sys
from contextlib import ExitStack

for _p in ("/opt/trn_rl_repo", "/root/.axon_site/_ro/trn_rl_repo"):
    if os.path.isdir(_p) and _p not in sys.path:
        sys.path.append(_p)

import numpy as np
import ml_dtypes

from concourse import bacc, tile, bass_utils, mybir
from concourse.bass import _add_dep_helper

BF16 = ml_dtypes.bfloat16

N = 8192          # nodes
D = 256           # width
NCORES = 8
ROWS = N // NCORES          # 1024 local rows
P = 128                     # partitions
IC = ROWS // P              # 8 local row chunks
JC = N // P                 # 64 contraction chunks
KPOLY = 10
NLAYERS = 2
SPLITS = (5, 3)             # row chunks per half-step gather
OFFS = (0, 5)

_BUILT = None


def _build():
    nc = bacc.Bacc("TRN2", target_bir_lowering=False, debug=False,
                   num_devices=NCORES)
    f32 = mybir.dt.float32
    bf = mybir.dt.bfloat16

    bp_d = nc.dram_tensor("bp", [N, ROWS], bf, kind="ExternalInput").ap()
    # X pre-shuffled into the gathered layout used by every step:
    # xg[h][r*128+p, q*256+d] = X[r*1024 + (OFFS[h]+q)*128 + p, d]
    xg0_d = nc.dram_tensor("xg0", [NCORES * P, SPLITS[0] * D], bf, kind="ExternalInput").ap()
    xg1_d = nc.dram_tensor("xg1", [NCORES * P, SPLITS[1] * D], bf, kind="ExternalInput").ap()
    xloc_d = nc.dram_tensor("xloc", [ROWS, D], bf, kind="ExternalInput").ap()
    xt_d = nc.dram_tensor("xt", [D, ROWS], bf, kind="ExternalInput").ap()
    w_d = nc.dram_tensor("w", [NLAYERS * KPOLY * 2, P, D], bf, kind="ExternalInput").ap()
    b_d = nc.dram_tensor("b", [NLAYERS, ROWS, D], f32, kind="ExternalInput").ap()
    id_d = nc.dram_tensor("ident", [P, P], bf, kind="ExternalInput").ap()
    out_d = nc.dram_tensor("out", [ROWS, D], f32, kind="ExternalOutput").ap()

    rg = [list(range(NCORES))]
    COPY = mybir.ActivationFunctionType.Copy
    TANH = mybir.ActivationFunctionType.Tanh
    MUL = mybir.AluOpType.mult
    SUB = mybir.AluOpType.subtract
    ADD = mybir.AluOpType.add

    with tile.TileContext(nc) as tc, ExitStack() as ctx:
        bppool = ctx.enter_context(tc.tile_pool(name="bp", bufs=JC))
        cstpool = ctx.enter_context(tc.tile_pool(name="cst", bufs=1))
        zlpool = ctx.enter_context(tc.tile_pool(name="zl", bufs=6))
        ztpool = ctx.enter_context(tc.tile_pool(name="zt", bufs=2))
        zspool = ctx.enter_context(tc.tile_pool(name="zs", bufs=5))
        tmppool = ctx.enter_context(tc.tile_pool(name="tmp", bufs=2))
        ocpool = ctx.enter_context(tc.tile_pool(name="oc", bufs=2))
        pspool = ctx.enter_context(tc.tile_pool(name="ps", bufs=4, space="PSUM"))
        ypool = ctx.enter_context(tc.tile_pool(name="y", bufs=1, space="PSUM"))
        dram = ctx.enter_context(tc.tile_pool(name="dram", bufs=8, space="DRAM"))

        # ---- constants / small residents (cheap; issued first) ----
        w_sb = cstpool.tile([P, NLAYERS * KPOLY * 2, D], bf, name="w_sb")
        nc.sync.dma_start(w_sb[:], w_d.rearrange("m p e -> p m e"))
        idn = cstpool.tile([P, P], bf, name="idn")
        nc.sync.dma_start(idn[:], id_d[:])
        zloc_prev1 = []
        for h in range(2):
            t = zlpool.tile([P, SPLITS[0], D], bf, name=f"zloc0_{h}", tag="zloc")
            nc.sync.dma_start(
                t[:, :SPLITS[h], :],
                xloc_d.rearrange("(c p) d -> p c d", p=P)[:, OFFS[h]:OFFS[h] + SPLITS[h], :])
            zloc_prev1.append(t)
        zt_cur = ztpool.tile([P, 2, ROWS], bf, name="xt0", tag="zt")
        nc.sync.dma_start(zt_cur[:], xt_d.rearrange("(dc p) i -> p dc i", p=P))

        # bp chunks are DMA'd on first use so the 16MB resident load paces
        # with the first step's matmul sweep instead of serializing ahead.
        bp_src = bp_d.rearrange("(c p) i -> p c i", p=P)
        bp_sb = {}

        def get_bp(jc):
            if jc not in bp_sb:
                t = bppool.tile([P, ROWS], bf, name=f"bp{jc}", tag="bp")
                nc.sync.dma_start(t[:], bp_src[:, jc, :])
                bp_sb[jc] = t
            return bp_sb[jc]

        b_sb_holder = []

        def get_b():
            if not b_sb_holder:
                t = cstpool.tile([P, NLAYERS, IC, D], f32, name="b_sb")
                nc.sync.dma_start(t[:], b_d.rearrange("l (c p) d -> p l c d", p=P))
                b_sb_holder.append(t)
            return b_sb_holder[0]

        def y_accum(Y, zt_t, l, k, ydeps, ics=range(IC)):
            # Y[:, ic, :] accumulates in pinned PSUM across the whole layer.
            # start clears has_written for a whole bank, so only the very
            # first matmul touching each bank (ic even, k==0, dc==0) sets it;
            # the odd-ic first matmul is ordered after it explicitly.
            for ic in ics:
                m = (l * KPOLY + k) * 2
                for dc in range(2):
                    mm = nc.tensor.matmul(
                        Y[:, ic, :], lhsT=zt_t[:, dc, ic * P:(ic + 1) * P],
                        rhs=w_sb[:, m + dc, :],
                        start=(k == 0 and dc == 0 and ic % 2 == 0),
                        stop=(k == KPOLY - 1 and dc == 1 and ic % 2 == 1),
                        skip_group_check=True)
                    if k == 0 and dc == 0:
                        if ic % 2 == 0:
                            ydeps[ic // 2] = mm
                        else:
                            _add_dep_helper(mm.ins, ydeps[ic // 2].ins, False,
                                            "bank-clear start runs first")

        def transpose_ics(zt_t, src_h, ics, tag=""):
            # [128,128] bf16 transposes on the PE (identity trick)
            for ic in ics:
                h = 0 if ic < SPLITS[0] else 1
                q = ic - OFFS[h]
                for dc in range(2):
                    ps = pspool.tile([P, P], bf, name=f"pstr{tag}_{ic}_{dc}",
                                     tag="ps")
                    nc.tensor.transpose(
                        ps[:], src_h[h][:, q, dc * P:(dc + 1) * P], idn[:])
                    nc.scalar.activation(zt_t[:, dc, ic * P:(ic + 1) * P], ps[:], COPY)

        def transpose_into(zt_t, src_h, l, k):
            transpose_ics(zt_t, src_h, range(IC))

        def gather(zloc_h, l, k, h):
            ns = SPLITS[h]
            agi = dram.tile([P, ns * D], bf, name=f"agi{l}_{k}_{h}", tag=f"agi{h}")
            nc.sync.dma_start(agi[:], zloc_h[:, :ns, :].rearrange("p c d -> p (c d)"))
            ago = dram.tile([NCORES * P, ns * D], bf, addr_space="Shared",
                            name=f"ago{l}_{k}_{h}", tag=f"ago{h}")
            nc.gpsimd.collective_compute(
                "AllGather", mybir.AluOpType.bypass, replica_groups=rg,
                ins=[agi[:].opt()], outs=[ago[:].opt()])
            return ago

        agout_prev = None  # layer 0 step 1 reads xg from DRAM directly
        zloc_prev2 = None

        for l in range(NLAYERS):
            Y = ypool.tile([P, IC, D], f32, name=f"y{l}", tag="y")
            ydeps = {}
            y_accum(Y, zt_cur, l, 0, ydeps)

            for k in range(1, KPOLY):
                if k == KPOLY - 2:
                    b_sb = get_b()
                zloc_k = [zlpool.tile([P, SPLITS[0], D], bf, name=f"zloc{l}_{k}_{h}",
                                      tag="zloc") for h in range(2)]
                zt_k = ztpool.tile([P, 2, ROWS], bf, name=f"zt{l}_{k}", tag="zt")
                if k == KPOLY - 1:
                    # layer tail is finalized per half so the boundary
                    # gathers/output overlap the second half's matmul sweep
                    if l == 0:
                        x1 = [zlpool.tile([P, SPLITS[0], D], bf, name=f"x1loc_{h}",
                                          tag="zloc") for h in range(2)]
                        xt1 = ztpool.tile([P, 2, ROWS], bf, name="xt1", tag="zt")
                agout_k = [None, None]
                for half in range(2):
                    ns = SPLITS[half]
                    npair = (ns + 1) // 2
                    ps = [pspool.tile([P, 2, D], f32, name=f"psr{l}_{k}_{half}_{t}",
                                      tag="ps") for t in range(npair)]
                    firstmm = {}
                    nmm = 0
                    for sh in range(2):
                        for r in range(NCORES):
                            zs = zspool.tile([P, SPLITS[0], D], bf,
                                             name=f"zs{l}_{k}_{half}_{sh}_{r}", tag="zs")
                            if l == 0 and k == 1:
                                src = (xg0_d if sh == 0 else xg1_d)[r * P:(r + 1) * P, :]
                            else:
                                src = agout_prev[sh][r * P:(r + 1) * P, :]
                            nc.sync.dma_start(
                                zs[:, :SPLITS[sh], :].rearrange("p c d -> p (c d)"), src)
                            for q in range(SPLITS[sh]):
                                jc = r * IC + OFFS[sh] + q
                                bp_t = get_bp(jc)
                                nmm += 1
                                lastjc = nmm == JC
                                for u in range(ns):
                                    ic = OFFS[half] + u
                                    t, lane = u // 2, u % 2
                                    st = t not in firstmm
                                    mm = nc.tensor.matmul(
                                        ps[t][:, lane, :],
                                        lhsT=bp_t[:, ic * P:(ic + 1) * P],
                                        rhs=zs[:, q, :],
                                        start=st,
                                        stop=(lastjc and u == min(2 * t + 1, ns - 1)),
                                        skip_group_check=True)
                                    if st:
                                        firstmm[t] = mm
                                    elif nmm == 1 and lane == 1:
                                        _add_dep_helper(mm.ins, firstmm[t].ins, False,
                                                        "bank-clear start runs first")
                    for u in range(ns):
                        t, lane = u // 2, u % 2
                        if k == 1:
                            nc.scalar.activation(zloc_k[half][:, u, :],
                                                 ps[t][:, lane, :], COPY)
                        else:
                            nc.vector.scalar_tensor_tensor(
                                out=zloc_k[half][:, u, :], in0=ps[t][:, lane, :],
                                scalar=2.0, in1=zloc_prev2[half][:, u, :],
                                op0=MUL, op1=SUB)
                    if k < KPOLY - 1:
                        agout_k[half] = gather(zloc_k[half], l, k, half)
                    else:
                        ics_h = range(OFFS[half], OFFS[half] + ns)
                        transpose_ics(zt_k, zloc_k, ics_h)
                        y_accum(Y, zt_k, l, k, ydeps, ics_h)
                        for ic in ics_h:
                            tmp = tmppool.tile([P, D], f32, name=f"pre{l}_{ic}",
                                               tag="tmp")
                            nc.vector.scalar_tensor_tensor(
                                out=tmp[:], in0=Y[:, ic, :], scalar=1.0,
                                in1=b_sb[:, l, ic, :], op0=MUL, op1=ADD)
                            if l == 0:
                                nc.scalar.activation(
                                    x1[half][:, ic - OFFS[half], :], tmp[:], TANH)
                            else:
                                oc = ocpool.tile([P, D], f32, name=f"oc{ic}", tag="oc")
                                nc.scalar.activation(oc[:], tmp[:], TANH)
                                nc.sync.dma_start(
                                    out_d.rearrange("(c p) d -> p c d", p=P)[:, ic, :],
                                    oc[:])
                        if l == 0:
                            transpose_ics(xt1, x1, ics_h)
                            agout_k[half] = gather(x1[half], l, 99, half)
                if k < KPOLY - 1:
                    transpose_into(zt_k, zloc_k, l, k)
                    y_accum(Y, zt_k, l, k, ydeps)
                zloc_prev2, zloc_prev1 = zloc_prev1, zloc_k
                agout_prev = agout_k

            if l == 0:
                zloc_prev1 = x1
                zloc_prev2 = None
                zt_cur = xt1

    nc.compile()
    return nc


def _get_nc():
    global _BUILT
    if _BUILT is None:
        _BUILT = _build()
    return _BUILT


def kernel(X, adj_mat, degree, W, b):
    X = np.asarray(X, dtype=np.float32)
    adj_mat = np.asarray(adj_mat, dtype=np.float32)
    degree = np.asarray(degree, dtype=np.float32)
    W = np.asarray(W, dtype=np.float32)
    b = np.asarray(b, dtype=np.float32)

    nc = _get_nc()

    xbf = X.astype(BF16)
    # gathered layouts: xg{h}[r*128+p, q*256+d] = X[r*1024 + (OFFS[h]+q)*128 + p, d]
    x4 = xbf.reshape(NCORES, IC, P, D)              # [r, c, p, d]
    xgs = []
    for h in range(2):
        sl = x4[:, OFFS[h]:OFFS[h] + SPLITS[h]]     # [r, q, p, d]
        xgs.append(np.ascontiguousarray(
            sl.transpose(0, 2, 1, 3).reshape(NCORES * P, SPLITS[h] * D)))
    ident = np.eye(P, dtype=BF16)
    wm = np.ascontiguousarray(
        W.reshape(NLAYERS * KPOLY, 2, P, D).reshape(NLAYERS * KPOLY * 2, P, D)
    ).astype(BF16)

    in_maps = []
    for r in range(NCORES):
        rows = slice(r * ROWS, (r + 1) * ROWS)
        lap_blk = (-adj_mat[rows] / degree[rows, None]).astype(BF16)   # [ROWS, N]
        bp = np.ascontiguousarray(lap_blk.T)                           # [N, ROWS]
        xloc = xbf[rows]
        in_maps.append({
            "bp": bp,
            "xg0": xgs[0],
            "xg1": xgs[1],
            "xloc": np.ascontiguousarray(xloc),
            "xt": np.ascontiguousarray(xloc.T),
            "w": wm,
            "b": np.ascontiguousarray(b[:, rows, :]),
            "ident": ident,
        })

    res = bass_utils.run_bass_kernel_spmd(
        nc, in_maps, core_ids=list(range(NCORES)),
        trace=bool(int(os.environ.get("CHEB_TRACE", "0"))))
    kernel.last_exec_time_ns = res.exec_time_ns
    out = np.concatenate([res.results[r]["out"] for r in range(NCORES)], axis=0)
    return out


kernel.last_exec_time_ns = None
